# revision 1
# baseline (speedup 1.0000x reference)
# Trainium2 Bass kernel for nn_Encoder (6-layer conv-attention encoder).
# Sharding: 4 batch groups x 2-way sequence split (cores 2g, 2g+1 own the two
# halves of batch g's sequence). k/v are computed redundantly for the full T
# on both cores of a pair; one AllGather exchanges the updated own-half of the
# residual stream per layer boundary.
import sys
sys.path.insert(0, '/opt/trn_rl_repo')
import numpy as np
import ml_dtypes

from concourse import bacc, tile, mybir
import concourse.bass as bass
from concourse.bass_utils import run_bass_kernel_spmd

B, C, T = 4, 512, 1024
F, KW, L, H = 2048, 3, 6, 8
KC, DR = 64, 32
TH, TE = 512, 516
NC8 = 8
BF16 = mybir.dt.bfloat16
F32 = mybir.dt.float32
U32 = mybir.dt.uint32
AF = mybir.ActivationFunctionType
ALU = mybir.AluOpType
EPS = 1e-4
P = 128

NCH_E = ((0, 512), (512, 4))      # extent-516 n-chunks (PSUM-bank aligned)
NCH_K = ((0, 512), (512, 512))    # extent-1024
NCH_V = ((0, 512), (512, 8))      # vT 520
NCH_H = ((0, 512), (512, 2))      # h 514

_CACHE = {}
TRACE = False
LAST_RESULT = None


def _emit(nc, tc, d, flags, n_layers=L, do_gather=True):
    (has_bv, ln1_aff, ln2_aff) = flags
    from contextlib import ExitStack
    ctx = ExitStack()

    def pool(name, bufs, space="SBUF"):
        return ctx.enter_context(tc.tile_pool(name=name, bufs=bufs, space=space))

    pers = pool("pers", 1)
    dram = pool("dram", 1, space="DRAM")
    p_psA = pool("psA", 2, space="PSUM")

    p_kraw = pool("kraw", 2)
    p_qraw = pool("qraw", 2)
    p_tmp = pool("ropetmp", 2)
    p_shuf = pool("shuf", 2)
    p_pt = pool("pt", 8)
    p_bc = pool("bc", 2)
    p_resid = pool("resid", 12)
    p_lntmp = pool("lntmp", 4)
    p_ht = pool("ht", 2)
    p_hm = pool("hm", 4)
    p_xrb = pool("xrb", 6)
    p_wq = pool("wq", 1)
    p_wk = pool("wk", 1)
    p_wv = pool("wv", 1)
    p_wo = pool("wo", 1)
    p_w1 = pool("w1", 3)
    p_w2 = pool("w2", 3)
    p_par = pool("par", 2)

    x_t = [pers.tile([P, T + 4], F32, tag=f"x{m}", name=f"x{m}") for m in range(4)]
    xb_t = [pers.tile([P, T], BF16, tag=f"xb{m}", name=f"xb{m}") for m in range(4)]
    xob_t = [pers.tile([P, TE], BF16, tag=f"xob{m}", name=f"xob{m}") for m in range(4)]
    xo_t = [pers.tile([P, TE], F32, tag=f"xo{m}", name=f"xo{m}") for m in range(4)]
    selab = pers.tile([P, 2], F32, tag="selab", name="selab")
    kr_t = [pers.tile([P, T], BF16, tag=f"kr{m}", name=f"kr{m}") for m in range(4)]
    qr_t = [pers.tile([P, TE], BF16, tag=f"qr{m}", name=f"qr{m}") for m in range(4)]
    vt_t = [pers.tile([P, 520], BF16, tag=f"vt{j}", name=f"vt{j}") for j in range(8)]
    onorm_t = [pers.tile([P, TE], BF16, tag=f"on{i}", name=f"on{i}") for i in range(4)]
    r2_t = [pers.tile([33, 520], BF16, tag=f"r2{i}", name=f"r2{i}") for i in range(4)]
    x1_t = [pers.tile([P, TE], F32, tag=f"x1{m}", name=f"x1{m}") for m in range(4)]
    SWAP_MASK = list(range(16, 32)) + list(range(0, 16))
    x1b_t = [pers.tile([P, TE], BF16, tag=f"x1b{m}", name=f"x1b{m}") for m in range(4)]
    cos_k = pers.tile([P, T], BF16, tag="cosk", name="cosk")
    sin_k = pers.tile([P, T], BF16, tag="sink", name="sink")
    cos_q = pers.tile([P, TE], BF16, tag="cosq", name="cosq")
    sin_q = pers.tile([P, TE], BF16, tag="sinq", name="sinq")
    maskx = pers.tile([P, TE], BF16, tag="maskx", name="maskx")
    maskh = pers.tile([P, 514], BF16, tag="maskh", name="maskh")
    sel2 = pers.tile([33, 128], BF16, tag="sel2", name="sel2")
    onesm = pers.tile([P, 256], BF16, tag="onesm", name="onesm")
    toff_sb = pers.tile([1, 1], U32, tag="toff", name="toff")
    eps_sb = pers.tile([P, 1], F32, tag="eps", name="eps")

    dma = nc.sync.dma_start
    for name, t in [("cos_k_d", cos_k), ("sin_k_d", sin_k), ("cos_q_d", cos_q),
                    ("sin_q_d", sin_q), ("maskx_d", maskx), ("maskh_d", maskh),
                    ("sel2_d", sel2), ("ones_d", onesm), ("toff_d", toff_sb),
                    ("selab_d", selab)]:
        dma(t[:, :], d[name][:, :])
    for m in range(4):
        nc.vector.memset(r2_t[m][:, :], 0.0)
        dma(x_t[m][:, :], d["x0_d"][m * P:(m + 1) * P, :])
    nc.vector.memset(eps_sb[:, :], EPS)


    def x_derived():
        for m in range(4):
            nc.vector.tensor_copy(xb_t[m][:, :], x_t[m][:, 2:2 + T])
            tsel = p_tmp.tile([P, TE], F32, tag="seltmp", name="seltmp")
            nc.vector.tensor_scalar_mul(tsel[:, 0:TE], x_t[m][:, 0:TE], selab[:, 0:1])
            nc.vector.scalar_tensor_tensor(xo_t[m][:, :], x_t[m][:, TH:TH + TE],
                                           selab[:, 1:2], tsel[:, 0:TE],
                                           op0=ALU.mult, op1=ALU.add)
            nc.vector.tensor_copy(xob_t[m][:, :], xo_t[m][:, :])

    x_derived()
    mm = nc.tensor.matmul

    def ln(xr_l, ext, nch, par, affcols, out_pers):
        """LayerNorm over channels. xr_l: 4 tiles [P, ext] f32. Returns out tiles."""
        xrb_l = []
        for m in range(4):
            xrb = p_xrb.tile([P, 520], BF16, tag="xrb", name="xrb")
            nc.vector.tensor_copy(xrb[:, 0:ext], xr_l[m][:, :])
            xrb_l.append(xrb)
        sum_ps = p_psA.tile([P, 1024], F32, tag="psA", name="psA")
        for (o, n) in nch:
            for kk in range(4):
                mm(sum_ps[:, o:o + n], onesm[:, 0:128], xrb_l[kk][:, o:o + n],
                   start=(kk == 0), stop=(kk == 3))
        sq_l = []
        for m in range(4):
            sq = p_xrb.tile([P, 520], BF16, tag="xrb", name="xrb")
            nc.scalar.activation(sq[:, 0:ext], xrb_l[m][:, 0:ext], AF.Square)
            sq_l.append(sq)
        sq_ps = p_psA.tile([P, 1024], F32, tag="psA", name="psA")
        for (o, n) in nch:
            for kk in range(4):
                mm(sq_ps[:, o:o + n], onesm[:, 128:256], sq_l[kk][:, o:o + n],
                   start=(kk == 0), stop=(kk == 3))
        mean2 = p_lntmp.tile([P, TE], F32, tag="lntmp", name="lntmp")
        nc.scalar.activation(mean2[:, 0:ext], sum_ps[:, 0:ext], AF.Square)
        var = p_lntmp.tile([P, TE], F32, tag="lntmp", name="lntmp")
        nc.vector.scalar_tensor_tensor(var[:, 0:ext], sq_ps[:, 0:ext], 1.0,
                                       mean2[:, 0:ext], op0=ALU.mult, op1=ALU.subtract)
        std = p_lntmp.tile([P, TE], F32, tag="lntmp", name="lntmp")
        nc.scalar.activation(std[:, 0:ext], var[:, 0:ext], AF.Sqrt, bias=eps_sb[:, 0:1])
        rstd = p_lntmp.tile([P, TE], F32, tag="lntmp", name="lntmp")
        nc.vector.reciprocal(rstd[:, 0:ext], std[:, 0:ext])
        out_l = []
        for m in range(4):
            dx = p_resid.tile([P, ext], F32, tag="resid", name="resid")
            nc.vector.tensor_add(dx[:, :], xr_l[m][:, :], sum_ps[:, 0:ext])
            if out_pers is None:
                xo = p_resid.tile([P, ext], F32, tag="resid", name="resid")
            else:
                xo = out_pers[m]
            nc.vector.tensor_mul(xo[:, 0:ext], dx[:, :], rstd[:, 0:ext])
            if affcols is not None:
                gc, bc_ = affcols
                nc.scalar.activation(xo[:, 0:ext], xo[:, 0:ext], AF.Identity,
                                     bias=par[:, bc_ + m:bc_ + m + 1],
                                     scale=par[:, gc + m:gc + m + 1])
            out_l.append(xo)
        return out_l

    for li in range(n_layers):
        last = li == n_layers - 1
        wq = p_wq.tile([P, 2048], BF16, tag="wq", name="wq")
        wk = p_wk.tile([P, 2048], BF16, tag="wk", name="wk")
        wv = p_wv.tile([P, 4 * 520], BF16, tag="wv", name="wv")
        wo = p_wo.tile([P, 2048], BF16, tag="wo", name="wo")
        par = p_par.tile([P, 52], F32, tag="par", name="par")
        for t, dn in [(wq, "wq_d"), (wk, "wk_d"), (wv, "wv_d"), (wo, "wo_d"), (par, "par_d")]:
            dma(t[:, :], d[dn][li][:, :])

        # ---- k projection + rope ----
        for m in range(4):
            ps = p_psA.tile([P, 1024], F32, tag="psA", name="psA")
            for (o, n) in NCH_K:
                for kk in range(4):
                    mm(ps[:, o:o + n], wk[:, kk * 512 + m * P: kk * 512 + (m + 1) * P],
                       xb_t[kk][:, o:o + n], start=(kk == 0), stop=(kk == 3))
            kraw = p_kraw.tile([P, T], BF16, tag="kraw", name="kraw")
            nc.scalar.activation(kraw[:, :], ps[:, 0:T], AF.Identity,
                                 bias=par[:, 4 + m:5 + m], scale=1.0)
            sh = p_shuf.tile([P, T], BF16, tag="shuf", name="shuf")
            nc.vector.stream_shuffle(sh[:, :], kraw[:, :], SWAP_MASK)
            t1 = p_tmp.tile([P, T], BF16, tag="ropetmp", name="ropetmp")
            t2 = p_tmp.tile([P, T], BF16, tag="ropetmp", name="ropetmp")
            nc.vector.tensor_mul(t1[:, :], kraw[:, :], cos_k[:, :])
            nc.vector.tensor_mul(t2[:, :], sh[:, :], sin_k[:, :])
            nc.vector.tensor_add(kr_t[m][:, :], t1[:, :], t2[:, :])

        # ---- q projection + rope ----
        for m in range(4):
            ps = p_psA.tile([P, 1024], F32, tag="psA", name="psA")
            for (o, n) in NCH_E:
                for kk in range(4):
                    mm(ps[:, o:o + n], wq[:, kk * 512 + m * P: kk * 512 + (m + 1) * P],
                       xob_t[kk][:, o:o + n], start=(kk == 0), stop=(kk == 3))
            qraw = p_qraw.tile([P, TE], BF16, tag="qraw", name="qraw")
            nc.scalar.activation(qraw[:, :], ps[:, 0:TE], AF.Identity,
                                 bias=par[:, 0 + m:1 + m], scale=1.0)
            sh = p_shuf.tile([P, T], BF16, tag="shuf", name="shuf")
            nc.vector.stream_shuffle(sh[:, 0:TE], qraw[:, :], SWAP_MASK)
            t1 = p_tmp.tile([P, T], BF16, tag="ropetmp", name="ropetmp")
            t2 = p_tmp.tile([P, T], BF16, tag="ropetmp", name="ropetmp")
            nc.vector.tensor_mul(t1[:, 0:TE], qraw[:, :], cos_q[:, :])
            nc.vector.tensor_mul(t2[:, 0:TE], sh[:, 0:TE], sin_q[:, :])
            nc.vector.tensor_add(qr_t[m][:, :], t1[:, 0:TE], t2[:, 0:TE])

        # ---- vT (x^T @ WvT, with ones column per head) ----
        for j in range(8):
            ps = p_psA.tile([P, 1024], F32, tag="psA", name="psA")
            for (o, n) in NCH_V:
                for kk in range(4):
                    mm(ps[:, o:o + n], xb_t[kk][:, j * P:(j + 1) * P],
                       wv[:, kk * 520 + o: kk * 520 + o + n],
                       start=(kk == 0), stop=(kk == 3))
            nc.scalar.activation(vt_t[j][:, :], ps[:, 0:520], AF.Copy)
            ones_ap = vt_t[j][:, :].rearrange("p (h c) -> p h c", c=65)[:, :, 64:65]
            nc.vector.memset(ones_ap, 1.0)

        # ---- attention ----
        with tc.tile_pool(name="psO", bufs=2, space="PSUM") as p_psO:
            for i in range(4):  # head pairs
                ops_pair = []
                for sub in range(2):
                    hh = 2 * i + sub
                    o_ps = p_psO.tile([65, TE], F32, tag="psO", name="psO")
                    for j in range(8):
                        sc = p_psA.tile([P, 1024], F32, tag="psA", name="psA")
                        for (o, n) in NCH_E:
                            mm(sc[:, o:o + n],
                               kr_t[i][sub * 64:(sub + 1) * 64, j * P:(j + 1) * P],
                               qr_t[i][sub * 64:(sub + 1) * 64, o:o + n],
                               start=True, stop=True)
                        pt = p_pt.tile([P, TE], BF16, tag="pt", name="pt")
                        nc.scalar.activation(pt[:, :], sc[:, 0:TE], AF.Exp)
                        for (o, n) in NCH_E:
                            mm(o_ps[:, o:o + n], vt_t[j][:, hh * 65:(hh + 1) * 65],
                               pt[:, o:o + n], start=(j == 0), stop=(j == 7),
                               skip_group_check=True)
                    with nc.allow_low_precision(reason="softmax 1/Z in bf16 is within tolerance"):
                        nc.vector.reciprocal(r2_t[i][sub * 32:sub * 32 + 1, 0:TE], o_ps[64:65, 0:TE])
                    ops_pair.append(o_ps)
                bc_ps = p_psA.tile([P, 1024], F32, tag="psA", name="psA")
                for (o, n) in NCH_E:
                    mm(bc_ps[:, o:o + n], sel2[:, :], r2_t[i][:, o:o + n],
                       start=True, stop=True)
                bc = p_bc.tile([P, TE], BF16, tag="bc", name="bc")
                nc.scalar.activation(bc[:, :], bc_ps[:, 0:TE], AF.Copy)
                for sub in range(2):
                    nc.vector.tensor_mul(onorm_t[i][sub * 64:(sub + 1) * 64, :],
                                         ops_pair[sub][0:64, 0:TE],
                                         bc[sub * 64:(sub + 1) * 64, :])
                    if has_bv:
                        nc.vector.tensor_scalar_add(
                            onorm_t[i][sub * 64:(sub + 1) * 64, :],
                            onorm_t[i][sub * 64:(sub + 1) * 64, :],
                            par[sub * 64:(sub + 1) * 64, 48 + i:49 + i])

        # ---- Wo + residual + LN1 ----
        xr_l = []
        for m in range(4):
            ps = p_psA.tile([P, 1024], F32, tag="psA", name="psA")
            for (o, n) in NCH_E:
                for kk in range(4):
                    mm(ps[:, o:o + n], wo[:, kk * 512 + m * P: kk * 512 + (m + 1) * P],
                       onorm_t[kk][:, o:o + n], start=(kk == 0), stop=(kk == 3))
            xr = p_resid.tile([P, TE], F32, tag="resid", name="resid")
            nc.vector.scalar_tensor_tensor(xr[:, :], ps[:, 0:TE], par[:, 8 + m:9 + m],
                                           xo_t[m][:, :],
                                           op0=ALU.add, op1=ALU.add)
            xr_l.append(xr)
        ln(xr_l, TE, NCH_E, par, (32, 36) if ln1_aff else None, x1_t)
        for m in range(4):
            nc.vector.tensor_mul(x1b_t[m][:, :], x1_t[m][:, :], maskx[:, :])

        # ---- FFN ----
        with tc.tile_pool(name="psY", bufs=4, space="PSUM") as p_psY:
            y_ps = [p_psY.tile([P, 512], F32, tag="psY", name="psY") for m in range(4)]
            for fm in range(16):
                w1t = p_w1.tile([P, 12 * 128], BF16, tag="w1", name="w1")
                dma(w1t[:, :], d["w1_d"][li][:, fm * 1536:(fm + 1) * 1536])
                h_ps = p_psA.tile([P, 1024], F32, tag="psA", name="psA")
                for (o, n) in NCH_H:
                    bidx = 0
                    for kk in range(4):
                        for dk in range(3):
                            mm(h_ps[:, o:o + n], w1t[:, bidx * 128:(bidx + 1) * 128],
                               x1b_t[kk][:, dk + o: dk + o + n],
                               start=(bidx == 0), stop=(bidx == 11))
                            bidx += 1
                ht = p_ht.tile([P, 514], BF16, tag="ht", name="ht")
                nc.scalar.activation(ht[:, :], h_ps[:, 0:514], AF.Relu,
                                     bias=par[:, 12 + fm:13 + fm], scale=1.0)
                hm = p_hm.tile([P, 514], BF16, tag="hm", name="hm")
                nc.vector.tensor_mul(hm[:, :], ht[:, :], maskh[:, :])
                for m in range(4):
                    w2t = p_w2.tile([P, 3 * 128], BF16, tag="w2", name="w2")
                    dma(w2t[:, :], d["w2_d"][li][:, (m * 48 + fm * 3) * 128:(m * 48 + fm * 3 + 3) * 128])
                    for dk in range(3):
                        mm(y_ps[m][:, 0:512], w2t[:, dk * 128:(dk + 1) * 128],
                           hm[:, dk:dk + 512],
                           start=(fm == 0 and dk == 0), stop=(fm == 15 and dk == 2),
                           skip_group_check=True)
            xr2_l = []
            for m in range(4):
                xr2 = p_resid.tile([P, TH], F32, tag="resid", name="resid")
                nc.vector.scalar_tensor_tensor(xr2[:, :], y_ps[m][:, 0:TH],
                                               par[:, 28 + m:29 + m],
                                               x1_t[m][:, 2:2 + TH],
                                               op0=ALU.add, op1=ALU.add)
                xr2_l.append(xr2)
            x2v = ln(xr2_l, TH, ((0, 512),), par, (40, 44) if ln2_aff else None, None)

        # ---- output / gather ----
        if last:
            for m in range(4):
                dma(d["out_d"][m * P:(m + 1) * P, :], x2v[m][:, :])
        elif not do_gather:
            for m in range(4):
                dma(x_t[m][:, 2:2 + TH], x2v[m][:, :])
            x_derived()
        else:
            bin_ = dram.tile([C, TH], F32, tag=f"bin{li}", name=f"bin{li}")
            bout = dram.tile([2 * C, TH], F32, tag=f"bout{li}", name=f"bout{li}")
            for m in range(4):
                dma(bin_[m * P:(m + 1) * P, :], x2v[m][:, :])
            nc.gpsimd.collective_compute(
                "AllGather", ALU.bypass,
                replica_groups=[[0, 1], [2, 3], [4, 5], [6, 7]],
                ins=[bin_[:, :].opt()], outs=[bout[:, :].opt()])
            for m in range(4):
                dma(x_t[m][:, 2:2 + TH], bout[m * P:(m + 1) * P, :])
                dma(x_t[m][:, 2 + TH:2 + T], bout[C + m * P: C + (m + 1) * P, :])
            x_derived()

    ctx.close()


def build_program(flags, n_layers=L, do_gather=True):
    nc = bacc.Bacc(target_bir_lowering=False, trn_type="TRN2", num_devices=NC8)
    d = {}
    d["x0_d"] = nc.declare_dram_parameter("x0", [C, T + 4], F32, isOutput=False)
    d["cos_k_d"] = nc.declare_dram_parameter("cos_k", [128, T], BF16, isOutput=False)
    d["sin_k_d"] = nc.declare_dram_parameter("sin_k", [128, T], BF16, isOutput=False)
    d["cos_q_d"] = nc.declare_dram_parameter("cos_q", [128, TE], BF16, isOutput=False)
    d["sin_q_d"] = nc.declare_dram_parameter("sin_q", [128, TE], BF16, isOutput=False)
    d["maskx_d"] = nc.declare_dram_parameter("maskx", [128, TE], BF16, isOutput=False)
    d["maskh_d"] = nc.declare_dram_parameter("maskh", [128, 514], BF16, isOutput=False)
    d["toff_d"] = nc.declare_dram_parameter("toff", [1, 1], U32, isOutput=False)
    d["selab_d"] = nc.declare_dram_parameter("selab", [128, 2], F32, isOutput=False)
    d["sel2_d"] = nc.declare_dram_parameter("sel2", [33, 128], BF16, isOutput=False)
    d["ones_d"] = nc.declare_dram_parameter("onesmat", [128, 256], BF16, isOutput=False)
    for key, shp, dt in [("wq_d", [128, 2048], BF16), ("wk_d", [128, 2048], BF16),
                         ("wv_d", [128, 4 * 520], BF16), ("wo_d", [128, 2048], BF16),
                         ("w1_d", [128, 16 * 12 * 128], BF16),
                         ("w2_d", [128, 4 * 48 * 128], BF16),
                         ("par_d", [128, 52], F32)]:
        d[key] = [nc.declare_dram_parameter(f"{key[:-2]}{i}", shp, dt, isOutput=False)
                  for i in range(L)]
    d["out_d"] = nc.declare_dram_parameter("out", [C, TH], F32, isOutput=True)
    with tile.TileContext(nc) as tc:
        _emit(nc, tc, d, flags, n_layers=n_layers, do_gather=do_gather)
    nc.compile()
    return nc


# ======================= host side =======================

def _rope_tables(tvals):
    theta = 1.0 / (10000.0 ** (np.arange(0, DR, 2) / DR))
    cos = np.ones((128, len(tvals)), np.float32)
    sin = np.zeros((128, len(tvals)), np.float32)
    for r in range(128):
        lc = r % 64
        if lc < 16:
            ang = theta[lc] * tvals
            cos[r] = np.cos(ang); sin[r] = -np.sin(ang)
        elif lc < 32:
            ang = theta[lc - 16] * tvals
            cos[r] = np.cos(ang); sin[r] = np.sin(ang)
    return cos, sin


def _bf(x):
    return np.ascontiguousarray(np.asarray(x, np.float32).astype(ml_dtypes.bfloat16))


def _pack_weights(inputs):
    per_layer = []
    for li in range(L):
        Wq = np.asarray(inputs['Wq'][li][:, :, 0], np.float32) / 8.0
        Wk = np.asarray(inputs['Wk'][li][:, :, 0], np.float32)
        Wv = np.asarray(inputs['Wv'][li][:, :, 0], np.float32)
        Wo = np.asarray(inputs['Wo'][li][:, :, 0], np.float32)
        W1 = np.asarray(inputs['W1'][li], np.float32)  # [F, C, 3]
        W2 = np.asarray(inputs['W2'][li], np.float32)  # [C, F, 3]

        def packT(W):
            WT = W.T
            return np.concatenate([WT[kk * 128:(kk + 1) * 128, :] for kk in range(4)], axis=1)

        wq_p = packT(Wq); wk_p = packT(Wk); wo_p = packT(Wo)
        WvT = Wv.T
        wv_p = np.zeros((128, 4 * 520), np.float32)
        for kk in range(4):
            blk = WvT[kk * 128:(kk + 1) * 128, :]
            for hh in range(8):
                wv_p[:, kk * 520 + hh * 65: kk * 520 + hh * 65 + 64] = blk[:, hh * 64:(hh + 1) * 64]
        w1_p = np.zeros((128, 16 * 12 * 128), np.float32)
        for fm in range(16):
            for kk in range(4):
                for dk in range(3):
                    b = kk * 3 + dk
                    w1_p[:, fm * 1536 + b * 128: fm * 1536 + (b + 1) * 128] = \
                        W1[fm * 128:(fm + 1) * 128, kk * 128:(kk + 1) * 128, dk].T
        w2_p = np.zeros((128, 4 * 48 * 128), np.float32)
        for m in range(4):
            for fk in range(16):
                for dk in range(3):
                    b = m * 48 + fk * 3 + dk
                    w2_p[:, b * 128:(b + 1) * 128] = \
                        W2[m * 128:(m + 1) * 128, fk * 128:(fk + 1) * 128, dk].T
        par = np.zeros((128, 52), np.float32)

        def col4(vec):
            return np.asarray(vec, np.float32).reshape(4, 128).T

        par[:, 0:4] = col4(inputs['bq'][li]) / 8.0
        par[:, 4:8] = col4(inputs['bk'][li])
        par[:, 8:12] = col4(inputs['bo'][li])
        par[:, 12:28] = np.asarray(inputs['c1'][li], np.float32).reshape(16, 128).T
        par[:, 28:32] = col4(inputs['c2'][li])
        par[:, 32:36] = col4(inputs['g1'][li])
        par[:, 36:40] = col4(inputs['be1'][li])
        par[:, 40:44] = col4(inputs['g2'][li])
        par[:, 44:48] = col4(inputs['be2'][li])
        par[:, 48:52] = col4(inputs['bv'][li])
        per_layer.append(dict(wq=_bf(wq_p), wk=_bf(wk_p), wv=_bf(wv_p), wo=_bf(wo_p),
                              w1=_bf(w1_p), w2=_bf(w2_p), par=par))
    return per_layer


def kernel(**inputs):
    inputs = {k: np.asarray(v) for k, v in inputs.items()}
    x = inputs['x'].astype(np.float32) * inputs['x_mask'].astype(np.float32)
    has_bv = bool(np.any(inputs['bv'] != 0))
    ln1_aff = bool(np.any(inputs['g1'] != 1) or np.any(inputs['be1'] != 0))
    ln2_aff = bool(np.any(inputs['g2'] != 1) or np.any(inputs['be2'] != 0))
    flags = (has_bv, ln1_aff, ln2_aff)
    if flags not in _CACHE:
        _CACHE[flags] = build_program(flags)
    nc = _CACHE[flags]

    wl = _pack_weights(inputs)
    cos_k, sin_k = _rope_tables(np.arange(T, dtype=np.float64))
    onesmat = np.concatenate([np.full((128, 128), -1.0 / 512, np.float32),
                              np.full((128, 128), 1.0 / 512, np.float32)], axis=1)
    sel2 = np.zeros((33, 128), np.float32)
    sel2[0, 0:64] = 1.0
    sel2[32, 64:128] = 1.0

    in_maps = []
    for core in range(NC8):
        g, h = core // 2, core % 2
        t0 = h * TH
        xp = np.zeros((C, T + 4), np.float32)
        xp[:, 2:2 + T] = x[g]
        cos_q, sin_q = _rope_tables(np.arange(t0 - 2, t0 + 514, dtype=np.float64))
        mx = np.ones((128, TE), np.float32)
        mh = np.ones((128, 514), np.float32)
        if h == 0:
            mx[:, 0:2] = 0; mh[:, 0:1] = 0
        else:
            mx[:, 514:516] = 0; mh[:, 513:514] = 0
        im = {
            "x0": xp,
            "cos_k": _bf(cos_k), "sin_k": _bf(sin_k),
            "cos_q": _bf(cos_q), "sin_q": _bf(sin_q),
            "maskx": _bf(mx), "maskh": _bf(mh),
            "toff": np.array([[t0]], np.uint32),
            "selab": np.repeat(np.array([[1.0 - h, float(h)]], np.float32), 128, axis=0),
            "sel2": _bf(sel2), "onesmat": _bf(onesmat),
        }
        for li in range(L):
            w = wl[li]
            im[f"wq{li}"] = w['wq']; im[f"wk{li}"] = w['wk']
            im[f"wv{li}"] = w['wv']; im[f"wo{li}"] = w['wo']
            im[f"w1{li}"] = w['w1']; im[f"w2{li}"] = w['w2']
            im[f"par{li}"] = w['par']
        in_maps.append(im)

    global LAST_RESULT
    res = run_bass_kernel_spmd(nc, in_maps, core_ids=list(range(NC8)),
                               trace=TRACE)
    LAST_RESULT = res
    out = np.zeros((B, C, T), np.float32)
    for g in range(B):
        out[g, :, 0:TH] = res.results[2 * g]["out"]
        out[g, :, TH:T] = res.results[2 * g + 1]["out"]
    out_dt = np.asarray(inputs['x']).dtype
    return out.astype(out_dt)



# revision 7
# speedup vs baseline: 1.1112x; 1.1112x over previous
# Trainium2 Bass kernel for nn_Encoder (6-layer conv-attention encoder).
# Sharding: 4 batch groups x 2-way sequence split. Each core owns one half of
# one batch element's sequence in "own coordinates" (cols = [t0-2, t0+514)).
# k/v are computed for the own half only and exchanged via one fused AllGather
# per layer; a tiny 4-column halo of the residual is exchanged per layer
# boundary. All SBUF data is fp16 (faster DVE modes, better precision than
# bf16); PSUM accumulation stays f32.
import sys
sys.path.insert(0, '/opt/trn_rl_repo')
import numpy as np

from concourse import bacc, tile, mybir
import concourse.bass as bass
from concourse.bass_utils import run_bass_kernel_spmd

B, C, T = 4, 512, 1024
F, KW, L, H = 2048, 3, 6, 8
KC, DR = 64, 32
TO, TE = 512, 516          # own cols / own+halo cols
NC8 = 8
F16 = mybir.dt.float16
F32 = mybir.dt.float32
AF = mybir.ActivationFunctionType
ALU = mybir.AluOpType
EPS = 1e-4
P = 128

NCH_E = ((0, 512), (512, 4))      # extent-516 psum-bank-aligned chunks
NCH_V = ((0, 512), (512, 8))      # vT 520

_CACHE = {}
TRACE = False
LAST_RESULT = None


def _emit(nc, tc, d, flags, n_layers=L, do_gather=True):
    (has_bv, ln1_aff, ln2_aff) = flags
    from contextlib import ExitStack
    ctx = ExitStack()

    def pool(name, bufs, space="SBUF"):
        return ctx.enter_context(tc.tile_pool(name=name, bufs=bufs, space=space))

    pers = pool("pers", 1)
    dram = pool("dram", 1, space="DRAM")
    p_psA = pool("psA", 2, space="PSUM")

    p_kraw = pool("kraw", 2)
    p_qraw = pool("qraw", 2)
    p_tmp = pool("ropetmp", 2)
    p_shuf = pool("shuf", 2)
    p_pt = pool("pt", 8)
    p_rbc = pool("rbc", 2)
    p_resid = pool("resid", 10)
    p_lntmp = pool("lntmp", 6)
    p_lndx = pool("lndx", 4)
    p_sq = pool("sq", 8)
    p_rstd = pool("rstd", 2)
    p_ht = pool("ht", 2)
    p_hm = pool("hm", 4)
    p_wq = pool("wq", 2)
    p_wk = pool("wk", 2)
    p_wv = pool("wv", 2)
    p_wo = pool("wo", 2)
    p_w1 = pool("w1", 3)
    p_w2 = pool("w2", 3)
    p_par = pool("par", 2)
    p_out = pool("outp", 1)
    p_halo = pool("halo", 2)

    x_t = [pers.tile([P, TE], F16, tag=f"x{m}", name=f"x{m}") for m in range(4)]
    kr_t = [pers.tile([P, T], F16, tag=f"kr{m}", name=f"kr{m}") for m in range(4)]
    q_t = [pers.tile([P, TE], F16, tag=f"q{m}", name=f"q{m}") for m in range(4)]
    ko_t = [pers.tile([P, TO], F16, tag=f"ko{m}", name=f"ko{m}") for m in range(4)]
    vt_t = [pers.tile([P, 520], F16, tag=f"vt{j}", name=f"vt{j}") for j in range(8)]
    vo_t = [pers.tile([P, 520], F16, tag=f"vo{j}", name=f"vo{j}") for j in range(4)]
    onorm_t = [pers.tile([P, TE], F16, tag=f"on{i}", name=f"on{i}") for i in range(4)]
    r2_t = [pers.tile([33, 520], F16, tag=f"r2{i}", name=f"r2{i}") for i in range(4)]
    x1b_t = [pers.tile([P, TE], F16, tag=f"x1b{m}", name=f"x1b{m}") for m in range(4)]
    SWAP_MASK = list(range(16, 32)) + list(range(0, 16))
    cos_k = pers.tile([P, TO], F16, tag="cosk", name="cosk")
    sin_k = pers.tile([P, TO], F16, tag="sink", name="sink")
    cos_q = pers.tile([P, TE], F16, tag="cosq", name="cosq")
    sin_q = pers.tile([P, TE], F16, tag="sinq", name="sinq")
    maskx = pers.tile([P, TE], F16, tag="maskx", name="maskx")
    maskh = pers.tile([P, 514], F16, tag="maskh", name="maskh")
    sel2 = pers.tile([33, 128], F16, tag="sel2", name="sel2")
    onesm = pers.tile([P, 256], F16, tag="onesm", name="onesm")
    eps_sb = pers.tile([P, 1], F32, tag="eps", name="eps")
    hcoef = pers.tile([P, 4], F32, tag="hcoef", name="hcoef")

    dma = nc.sync.dma_start
    for name, t in [("cos_k_d", cos_k), ("sin_k_d", sin_k), ("cos_q_d", cos_q),
                    ("sin_q_d", sin_q), ("maskx_d", maskx), ("maskh_d", maskh),
                    ("sel2_d", sel2), ("ones_d", onesm), ("hcoef_d", hcoef)]:
        dma(t[:, :], d[name][:, :])
    for m in range(4):
        dma(x_t[m][:, :], d["x0_d"][m * P:(m + 1) * P, :])
        nc.vector.memset(r2_t[m][:, :], 0.0)
    nc.vector.memset(eps_sb[:, :], EPS)

    mm = nc.tensor.matmul

    def mm_chunks(o, n):
        """Split a column range at the 512 psum-bank boundary."""
        out = []
        if o < 512:
            out.append((o, min(n, 512 - o)))
        if o + n > 512:
            oo = max(o, 512)
            out.append((oo, o + n - oo))
        return out

    def ln(xr_l, ext, chunks, par, affcols, out_l, rstd_mask=None, out_off=0):
        """Channel LayerNorm. xr_l: 4 [P, ext] f16 tiles. Writes out_l tiles
        at column offset out_off. chunks: column ranges processed as
        independent pipelined chains."""
        sum_ps = p_psA.tile([P, 1024], F32, tag="psA", name="psA")
        sq_ps = p_psA.tile([P, 1024], F32, tag="psA", name="psA")
        for (o, n) in chunks:
            for (oo, nn) in mm_chunks(o, n):
                for kk in range(4):
                    mm(sum_ps[:, oo:oo + nn], onesm[:, 0:128], xr_l[kk][:, oo:oo + nn],
                       start=(kk == 0), stop=(kk == 3), skip_group_check=True)
            sq_l = []
            for kk in range(4):
                sq = p_sq.tile([P, 520], F16, tag="sq", name="sq")
                nc.vector.tensor_mul(sq[:, o:o + n], xr_l[kk][:, o:o + n],
                                     xr_l[kk][:, o:o + n])
                sq_l.append(sq)
            for (oo, nn) in mm_chunks(o, n):
                for kk in range(4):
                    mm(sq_ps[:, oo:oo + nn], onesm[:, 128:256], sq_l[kk][:, oo:oo + nn],
                       start=(kk == 0), stop=(kk == 3), skip_group_check=True)
            mean2 = p_lntmp.tile([P, TE], F32, tag="lntmp", name="lntmp")
            nc.scalar.activation(mean2[:, o:o + n], sum_ps[:, o:o + n], AF.Square)
            var = p_lntmp.tile([P, TE], F32, tag="lntmp", name="lntmp")
            nc.vector.scalar_tensor_tensor(var[:, o:o + n], sq_ps[:, o:o + n], 1.0,
                                           mean2[:, o:o + n],
                                           op0=ALU.mult, op1=ALU.subtract)
            lnv = p_lntmp.tile([P, TE], F32, tag="lntmp", name="lntmp")
            nc.scalar.activation(lnv[:, o:o + n], var[:, o:o + n], AF.Ln,
                                 bias=eps_sb[:, 0:1])
            rstd = p_rstd.tile([P, TE], F16, tag="rstd", name="rstd")
            nc.scalar.activation(rstd[:, o:o + n], lnv[:, o:o + n], AF.Exp, scale=-0.5)
            if rstd_mask is not None:
                nc.vector.tensor_mul(rstd[:, o:o + n], rstd[:, o:o + n],
                                     rstd_mask[:, o:o + n])
            for m in range(4):
                dx = p_lndx.tile([P, TE], F16, tag="lndx", name="lndx")
                nc.vector.tensor_add(dx[:, o:o + n], xr_l[m][:, o:o + n],
                                     sum_ps[:, o:o + n])
                oap = out_l[m][:, out_off + o:out_off + o + n]
                nc.vector.tensor_mul(oap, dx[:, o:o + n], rstd[:, o:o + n])
                if affcols is not None:
                    gc, bc_ = affcols
                    nc.scalar.activation(oap, out_l[m][:, out_off + o:out_off + o + n],
                                         AF.Identity, bias=par[:, bc_ + m:bc_ + m + 1],
                                         scale=par[:, gc + m:gc + m + 1])

    for li in range(n_layers):
        last = li == n_layers - 1
        wq = p_wq.tile([P, 2048], F16, tag="wq", name="wq")
        wk = p_wk.tile([P, 2048], F16, tag="wk", name="wk")
        wv = p_wv.tile([P, 4 * 520], F16, tag="wv", name="wv")
        wo = p_wo.tile([P, 2048], F16, tag="wo", name="wo")
        par = p_par.tile([P, 52], F32, tag="par", name="par")
        for t, dn in [(wk, "wk_d"), (wv, "wv_d"), (wq, "wq_d"), (wo, "wo_d"), (par, "par_d")]:
            dma(t[:, :], d[dn][li][:, :])

        bin_kv = dram.tile([C, 1032], F16, tag=f"bkv{li}", name=f"bkv{li}")
        bout_kv = dram.tile([2 * C, 1032], F16, tag=f"bokv{li}", name=f"bokv{li}")

        # ---- k own-half projection + rope ----
        for m in range(4):
            ps = p_psA.tile([P, 1024], F32, tag="psA", name="psA")
            for kk in range(4):
                mm(ps[:, 0:TO], wk[:, kk * 512 + m * P: kk * 512 + (m + 1) * P],
                   x_t[kk][:, 2:2 + TO], start=(kk == 0), stop=(kk == 3))
            kraw = p_kraw.tile([P, TO], F16, tag="kraw", name="kraw")
            nc.scalar.activation(kraw[:, :], ps[:, 0:TO], AF.Identity,
                                 bias=par[:, 4 + m:5 + m], scale=1.0)
            sh = p_shuf.tile([P, TO], F16, tag="shuf", name="shuf")
            nc.vector.stream_shuffle(sh[:, :], kraw[:, :], SWAP_MASK)
            t1 = p_tmp.tile([P, TO], F16, tag="ropetmp", name="ropetmp")
            t2 = p_tmp.tile([P, TO], F16, tag="ropetmp", name="ropetmp")
            nc.vector.tensor_mul(t1[:, :], kraw[:, :], cos_k[:, :])
            nc.vector.tensor_mul(t2[:, :], sh[:, :], sin_k[:, :])
            nc.vector.tensor_add(ko_t[m][:, :], t1[:, :], t2[:, :])
            dma(bin_kv[m * P:(m + 1) * P, 0:512], ko_t[m][:, :])

        # ---- v own-half (transposed, with ones column per head) ----
        for jj in range(4):
            ps = p_psA.tile([P, 1024], F32, tag="psA", name="psA")
            for (o, n) in NCH_V:
                for kk in range(4):
                    mm(ps[:, o:o + n], x_t[kk][:, 2 + jj * P:2 + (jj + 1) * P],
                       wv[:, kk * 520 + o: kk * 520 + o + n],
                       start=(kk == 0), stop=(kk == 3))
            nc.scalar.activation(vo_t[jj][:, :], ps[:, 0:520], AF.Copy)
            ones_ap = vo_t[jj][:, :].rearrange("p (h c) -> p h c", c=65)[:, :, 64:65]
            nc.vector.memset(ones_ap, 1.0)
            dma(bin_kv[jj * P:(jj + 1) * P, 512:1032], vo_t[jj][:, :])

        # ---- k/v exchange (own half <-> pair partner) ----
        if do_gather:
            nc.gpsimd.collective_compute(
                "AllGather", ALU.bypass,
                replica_groups=[[0, 1], [2, 3], [4, 5], [6, 7]],
                ins=[bin_kv[:, :].opt()], outs=[bout_kv[:, :].opt()])
            kv_src = bout_kv
        else:
            kv_src = bin_kv
        for m in range(4):
            dma(kr_t[m][:, 0:512], kv_src[m * P:(m + 1) * P, 0:512])
            if do_gather:
                dma(kr_t[m][:, 512:1024], kv_src[C + m * P:C + (m + 1) * P, 0:512])
            else:
                dma(kr_t[m][:, 512:1024], kv_src[m * P:(m + 1) * P, 0:512])
        for j in range(8):
            jj = j % 4
            src_r = jj * P if (j < 4 or not do_gather) else C + jj * P
            dma(vt_t[j][:, :], kv_src[src_r:src_r + P, 512:1032])

        # ---- q projection + rope (overlaps the k/v exchange) ----
        for m in range(4):
            ps = p_psA.tile([P, 1024], F32, tag="psA", name="psA")
            for (o, n) in NCH_E:
                for kk in range(4):
                    mm(ps[:, o:o + n], wq[:, kk * 512 + m * P: kk * 512 + (m + 1) * P],
                       x_t[kk][:, o:o + n], start=(kk == 0), stop=(kk == 3))
            qraw = p_qraw.tile([P, TE], F16, tag="qraw", name="qraw")
            nc.scalar.activation(qraw[:, :], ps[:, 0:TE], AF.Identity,
                                 bias=par[:, 0 + m:1 + m], scale=1.0)
            sh = p_shuf.tile([P, TE], F16, tag="shuf", name="shuf")
            nc.vector.stream_shuffle(sh[:, :], qraw[:, :], SWAP_MASK)
            t1 = p_tmp.tile([P, TE], F16, tag="ropetmp", name="ropetmp")
            t2 = p_tmp.tile([P, TE], F16, tag="ropetmp", name="ropetmp")
            nc.vector.tensor_mul(t1[:, :], qraw[:, :], cos_q[:, :])
            nc.vector.tensor_mul(t2[:, :], sh[:, :], sin_q[:, :])
            nc.vector.tensor_add(q_t[m][:, :], t1[:, :], t2[:, :])

        # ---- attention ----
        with tc.tile_pool(name="psO", bufs=2, space="PSUM") as p_psO:
            for i in range(4):  # head pairs
                ops_pair = []
                for sub in range(2):
                    hh = 2 * i + sub
                    o_ps = p_psO.tile([65, TE], F32, tag="psO", name="psO")
                    for j in range(8):
                        sc = p_psA.tile([P, 1024], F32, tag="psA", name="psA")
                        for (o, n) in NCH_E:
                            mm(sc[:, o:o + n],
                               kr_t[i][sub * 64:(sub + 1) * 64, j * P:(j + 1) * P],
                               q_t[i][sub * 64:(sub + 1) * 64, o:o + n],
                               start=True, stop=True)
                        pt = p_pt.tile([P, TE], F16, tag="pt", name="pt")
                        nc.scalar.activation(pt[:, :], sc[:, 0:TE], AF.Exp)
                        for (o, n) in NCH_E:
                            mm(o_ps[:, o:o + n], vt_t[j][:, hh * 65:(hh + 1) * 65],
                               pt[:, o:o + n], start=(j == 0), stop=(j == 7),
                               skip_group_check=True)
                    nc.vector.tensor_copy(r2_t[i][sub * 32:sub * 32 + 1, 0:TE],
                                          o_ps[64:65, 0:TE])
                    ops_pair.append(o_ps)
                bc_ps = p_psA.tile([P, 1024], F32, tag="psA", name="psA")
                for (o, n) in NCH_E:
                    mm(bc_ps[:, o:o + n], sel2[:, :], r2_t[i][:, o:o + n],
                       start=True, stop=True)
                rbc = p_rbc.tile([P, TE], F16, tag="rbc", name="rbc")
                with nc.allow_low_precision(reason="softmax 1/Z in fp16 is within tolerance"):
                    nc.vector.reciprocal(rbc[:, :], bc_ps[:, 0:TE])
                for sub in range(2):
                    nc.vector.tensor_mul(onorm_t[i][sub * 64:(sub + 1) * 64, :],
                                         ops_pair[sub][0:64, 0:TE],
                                         rbc[sub * 64:(sub + 1) * 64, :])
                    if has_bv:
                        nc.vector.tensor_scalar_add(
                            onorm_t[i][sub * 64:(sub + 1) * 64, :],
                            onorm_t[i][sub * 64:(sub + 1) * 64, :],
                            par[sub * 64:(sub + 1) * 64, 48 + i:49 + i])

        # ---- Wo + residual + LN1 ----
        xr_l = []
        for m in range(4):
            ps = p_psA.tile([P, 1024], F32, tag="psA", name="psA")
            for (o, n) in NCH_E:
                for kk in range(4):
                    mm(ps[:, o:o + n], wo[:, kk * 512 + m * P: kk * 512 + (m + 1) * P],
                       onorm_t[kk][:, o:o + n], start=(kk == 0), stop=(kk == 3))
            xr = p_resid.tile([P, TE], F16, tag="resid", name="resid")
            nc.vector.scalar_tensor_tensor(xr[:, :], ps[:, 0:TE], par[:, 8 + m:9 + m],
                                           x_t[m][:, :], op0=ALU.add, op1=ALU.add)
            xr_l.append(xr)
        ln(xr_l, TE, ((0, 260), (260, 256)), par, (32, 36) if ln1_aff else None,
           x1b_t, rstd_mask=maskx)

        # ---- FFN ----
        HCH = ((0, 258), (258, 256))  # h extent 514, pipelined halves
        with tc.tile_pool(name="psY", bufs=4, space="PSUM") as p_psY:
            y_ps = [p_psY.tile([P, 512], F32, tag="psY", name="psY") for m in range(4)]
            for fm in range(16):
                w1t = p_w1.tile([P, 12 * 128], F16, tag="w1", name="w1")
                dma(w1t[:, :], d["w1_d"][li][:, fm * 1536:(fm + 1) * 1536])
                h_ps = p_psA.tile([P, 1024], F32, tag="psA", name="psA")
                for (o, n) in HCH:
                    for (oo, nn) in mm_chunks(o, n):
                        bidx = 0
                        for kk in range(4):
                            for dk in range(3):
                                mm(h_ps[:, oo:oo + nn], w1t[:, bidx * 128:(bidx + 1) * 128],
                                   x1b_t[kk][:, dk + oo: dk + oo + nn],
                                   start=(bidx == 0), stop=(bidx == 11),
                                   skip_group_check=True)
                                bidx += 1
                ht = p_ht.tile([P, 514], F16, tag="ht", name="ht")
                nc.scalar.activation(ht[:, :], h_ps[:, 0:514], AF.Relu,
                                     bias=par[:, 12 + fm:13 + fm], scale=1.0)
                hm = p_hm.tile([P, 514], F16, tag="hm", name="hm")
                nc.vector.tensor_mul(hm[:, :], ht[:, :], maskh[:, :])
                w2t = p_w2.tile([P, 12 * 128], F16, tag="w2", name="w2")
                dma(w2t[:, :], d["w2_d"][li][:, fm * 1536:(fm + 1) * 1536])
                for m in range(4):
                    for dk in range(3):
                        mm(y_ps[m][:, 0:512], w2t[:, (m * 3 + dk) * 128:(m * 3 + dk + 1) * 128],
                           hm[:, dk:dk + 512],
                           start=(fm == 0 and dk == 0), stop=(fm == 15 and dk == 2),
                           skip_group_check=True)
            xr2_l = []
            for m in range(4):
                xr2 = p_resid.tile([P, TO], F16, tag="resid", name="resid")
                nc.vector.scalar_tensor_tensor(xr2[:, :], y_ps[m][:, 0:TO],
                                               par[:, 28 + m:29 + m],
                                               x1b_t[m][:, 2:2 + TO],
                                               op0=ALU.add, op1=ALU.add)
                xr2_l.append(xr2)
            if last:
                o32 = [p_out.tile([P, TO], F32, tag=f"o32{m}", name=f"o32{m}")
                       for m in range(4)]
                ln(xr2_l, TO, ((0, 258), (258, 254)), par,
                   (40, 44) if ln2_aff else None, o32)
                for m in range(4):
                    dma(d["out_d"][m * P:(m + 1) * P, :], o32[m][:, :])
            else:
                ln(xr2_l, TO, ((0, 258), (258, 254)), par,
                   (40, 44) if ln2_aff else None, x_t, out_off=2)

        # ---- residual halo exchange (4 boundary cols) ----
        if not last:
            bin_h = dram.tile([C, 4], F16, tag=f"bh{li}", name=f"bh{li}")
            bout_h = dram.tile([2 * C, 4], F16, tag=f"boh{li}", name=f"boh{li}")
            for m in range(4):
                dma(bin_h[m * P:(m + 1) * P, 0:2], x_t[m][:, 2:4])
                dma(bin_h[m * P:(m + 1) * P, 2:4], x_t[m][:, 512:514])
            if do_gather:
                nc.gpsimd.collective_compute(
                    "AllGather", ALU.bypass,
                    replica_groups=[[0, 1], [2, 3], [4, 5], [6, 7]],
                    ins=[bin_h[:, :].opt()], outs=[bout_h[:, :].opt()])
                h_src = bout_h
            else:
                h_src = bin_h
            for m in range(4):
                hA = p_halo.tile([P, 4], F16, tag="halo", name="halo")
                hB = p_halo.tile([P, 4], F16, tag="halo", name="halo")
                dma(hA[:, :], h_src[m * P:(m + 1) * P, :])
                if do_gather:
                    dma(hB[:, :], h_src[C + m * P:C + (m + 1) * P, :])
                else:
                    dma(hB[:, :], h_src[m * P:(m + 1) * P, :])
                # left halo cols 0:2  = hA.last2*cl0 + hB.last2*cl1
                tl = p_halo.tile([P, 4], F16, tag="halot", name="halot")
                nc.vector.tensor_scalar_mul(tl[:, 0:2], hA[:, 2:4], hcoef[:, 0:1])
                nc.vector.scalar_tensor_tensor(x_t[m][:, 0:2], hB[:, 2:4],
                                               hcoef[:, 1:2], tl[:, 0:2],
                                               op0=ALU.mult, op1=ALU.add)
                # right halo cols 514:516 = hA.first2*cr0 + hB.first2*cr1
                nc.vector.tensor_scalar_mul(tl[:, 2:4], hA[:, 0:2], hcoef[:, 2:3])
                nc.vector.scalar_tensor_tensor(x_t[m][:, 514:516], hB[:, 0:2],
                                               hcoef[:, 3:4], tl[:, 2:4],
                                               op0=ALU.mult, op1=ALU.add)

    ctx.close()


def build_program(flags, n_layers=L, do_gather=True):
    nc = bacc.Bacc(target_bir_lowering=False, trn_type="TRN2", num_devices=NC8)
    d = {}
    d["x0_d"] = nc.declare_dram_parameter("x0", [C, TE], F16, isOutput=False)
    d["cos_k_d"] = nc.declare_dram_parameter("cos_k", [128, TO], F16, isOutput=False)
    d["sin_k_d"] = nc.declare_dram_parameter("sin_k", [128, TO], F16, isOutput=False)
    d["cos_q_d"] = nc.declare_dram_parameter("cos_q", [128, TE], F16, isOutput=False)
    d["sin_q_d"] = nc.declare_dram_parameter("sin_q", [128, TE], F16, isOutput=False)
    d["maskx_d"] = nc.declare_dram_parameter("maskx", [128, TE], F16, isOutput=False)
    d["maskh_d"] = nc.declare_dram_parameter("maskh", [128, 514], F16, isOutput=False)
    d["hcoef_d"] = nc.declare_dram_parameter("hcoef", [128, 4], F32, isOutput=False)
    d["sel2_d"] = nc.declare_dram_parameter("sel2", [33, 128], F16, isOutput=False)
    d["ones_d"] = nc.declare_dram_parameter("onesmat", [128, 256], F16, isOutput=False)
    for key, shp, dt in [("wq_d", [128, 2048], F16), ("wk_d", [128, 2048], F16),
                         ("wv_d", [128, 4 * 520], F16), ("wo_d", [128, 2048], F16),
                         ("w1_d", [128, 16 * 12 * 128], F16),
                         ("w2_d", [128, 16 * 12 * 128], F16),
                         ("par_d", [128, 52], F32)]:
        d[key] = [nc.declare_dram_parameter(f"{key[:-2]}{i}", shp, dt, isOutput=False)
                  for i in range(L)]
    d["out_d"] = nc.declare_dram_parameter("out", [C, TO], F32, isOutput=True)
    with tile.TileContext(nc) as tc:
        _emit(nc, tc, d, flags, n_layers=n_layers, do_gather=do_gather)
    nc.compile()
    return nc


# ======================= host side =======================

def _rope_tables(tvals):
    theta = 1.0 / (10000.0 ** (np.arange(0, DR, 2) / DR))
    cos = np.ones((128, len(tvals)), np.float32)
    sin = np.zeros((128, len(tvals)), np.float32)
    for r in range(128):
        lc = r % 64
        if lc < 16:
            ang = theta[lc] * tvals
            cos[r] = np.cos(ang); sin[r] = -np.sin(ang)
        elif lc < 32:
            ang = theta[lc - 16] * tvals
            cos[r] = np.cos(ang); sin[r] = np.sin(ang)
    return cos, sin


def _f16(x):
    return np.ascontiguousarray(np.asarray(x, np.float32).astype(np.float16))


def _pack_weights(inputs):
    per_layer = []
    for li in range(L):
        Wq = np.asarray(inputs['Wq'][li][:, :, 0], np.float32) / 8.0
        Wk = np.asarray(inputs['Wk'][li][:, :, 0], np.float32)
        Wv = np.asarray(inputs['Wv'][li][:, :, 0], np.float32)
        Wo = np.asarray(inputs['Wo'][li][:, :, 0], np.float32)
        W1 = np.asarray(inputs['W1'][li], np.float32)  # [F, C, 3]
        W2 = np.asarray(inputs['W2'][li], np.float32)  # [C, F, 3]

        def packT(W):
            WT = W.T
            return np.concatenate([WT[kk * 128:(kk + 1) * 128, :] for kk in range(4)], axis=1)

        wq_p = packT(Wq); wk_p = packT(Wk); wo_p = packT(Wo)
        WvT = Wv.T
        wv_p = np.zeros((128, 4 * 520), np.float32)
        for kk in range(4):
            blk = WvT[kk * 128:(kk + 1) * 128, :]
            for hh in range(8):
                wv_p[:, kk * 520 + hh * 65: kk * 520 + hh * 65 + 64] = blk[:, hh * 64:(hh + 1) * 64]
        w1_p = np.zeros((128, 16 * 12 * 128), np.float32)
        for fm in range(16):
            for kk in range(4):
                for dk in range(3):
                    b = kk * 3 + dk
                    w1_p[:, fm * 1536 + b * 128: fm * 1536 + (b + 1) * 128] = \
                        W1[fm * 128:(fm + 1) * 128, kk * 128:(kk + 1) * 128, dk].T
        w2_p = np.zeros((128, 16 * 12 * 128), np.float32)
        for fk in range(16):
            for m in range(4):
                for dk in range(3):
                    b = fk * 12 + m * 3 + dk
                    w2_p[:, b * 128:(b + 1) * 128] = \
                        W2[m * 128:(m + 1) * 128, fk * 128:(fk + 1) * 128, dk].T
        par = np.zeros((128, 52), np.float32)

        def col4(vec):
            return np.asarray(vec, np.float32).reshape(4, 128).T

        par[:, 0:4] = col4(inputs['bq'][li]) / 8.0
        par[:, 4:8] = col4(inputs['bk'][li])
        par[:, 8:12] = col4(inputs['bo'][li])
        par[:, 12:28] = np.asarray(inputs['c1'][li], np.float32).reshape(16, 128).T
        par[:, 28:32] = col4(inputs['c2'][li])
        par[:, 32:36] = col4(inputs['g1'][li])
        par[:, 36:40] = col4(inputs['be1'][li])
        par[:, 40:44] = col4(inputs['g2'][li])
        par[:, 44:48] = col4(inputs['be2'][li])
        par[:, 48:52] = col4(inputs['bv'][li])
        per_layer.append(dict(wq=_f16(wq_p), wk=_f16(wk_p), wv=_f16(wv_p),
                              wo=_f16(wo_p), w1=_f16(w1_p), w2=_f16(w2_p), par=par))
    return per_layer


def kernel(**inputs):
    inputs = {k: np.asarray(v) for k, v in inputs.items()}
    x = inputs['x'].astype(np.float32) * inputs['x_mask'].astype(np.float32)
    has_bv = bool(np.any(inputs['bv'] != 0))
    ln1_aff = bool(np.any(inputs['g1'] != 1) or np.any(inputs['be1'] != 0))
    ln2_aff = bool(np.any(inputs['g2'] != 1) or np.any(inputs['be2'] != 0))
    flags = (has_bv, ln1_aff, ln2_aff)
    if flags not in _CACHE:
        _CACHE[flags] = build_program(flags)
    nc = _CACHE[flags]

    wl = _pack_weights(inputs)
    onesmat = np.concatenate([np.full((128, 128), -1.0 / 512, np.float32),
                              np.full((128, 128), 1.0 / 512, np.float32)], axis=1)
    sel2 = np.zeros((33, 128), np.float32)
    sel2[0, 0:64] = 1.0
    sel2[32, 64:128] = 1.0

    in_maps = []
    for core in range(NC8):
        g, h = core // 2, core % 2
        t0 = h * TO
        # own+halo window [t0-2, t0+514), zero-padded at sequence edges
        xp = np.zeros((C, TE), np.float32)
        lo, hi = max(t0 - 2, 0), min(t0 + 514, T)
        xp[:, lo - (t0 - 2):hi - (t0 - 2)] = x[g][:, lo:hi]
        cos_k, sin_k = _rope_tables(np.arange(t0, t0 + TO, dtype=np.float64))
        cos_q, sin_q = _rope_tables(np.arange(t0 - 2, t0 + 514, dtype=np.float64))
        mx = np.ones((128, TE), np.float32)
        mh = np.ones((128, 514), np.float32)
        if h == 0:
            mx[:, 0:2] = 0; mh[:, 0:1] = 0
            hc = np.array([0.0, 0.0, 0.0, 1.0], np.float32)   # cl0, cl1, cr0, cr1
        else:
            mx[:, 514:516] = 0; mh[:, 513:514] = 0
            hc = np.array([1.0, 0.0, 0.0, 0.0], np.float32)
        im = {
            "x0": _f16(xp),
            "cos_k": _f16(cos_k), "sin_k": _f16(sin_k),
            "cos_q": _f16(cos_q), "sin_q": _f16(sin_q),
            "maskx": _f16(mx), "maskh": _f16(mh),
            "hcoef": np.repeat(hc[None, :], 128, axis=0),
            "sel2": _f16(sel2), "onesmat": _f16(onesmat),
        }
        for li in range(L):
            w = wl[li]
            im[f"wq{li}"] = w['wq']; im[f"wk{li}"] = w['wk']
            im[f"wv{li}"] = w['wv']; im[f"wo{li}"] = w['wo']
            im[f"w1{li}"] = w['w1']; im[f"w2{li}"] = w['w2']
            im[f"par{li}"] = w['par']
        in_maps.append(im)

    global LAST_RESULT
    res = run_bass_kernel_spmd(nc, in_maps, core_ids=list(range(NC8)),
                               trace=TRACE)
    LAST_RESULT = res
    out = np.zeros((B, C, T), np.float32)
    for g in range(B):
        out[g, :, 0:TO] = res.results[2 * g]["out"]
        out[g, :, TO:T] = res.results[2 * g + 1]["out"]
    out_dt = np.asarray(inputs['x']).dtype
    return out.astype(out_dt)
